# revision 6
# baseline (speedup 1.0000x reference)
"""Trainium2 Bass kernel for nn_CrossAttention_68350109549162.

Math (see reference): the single K/V token makes attention softmax trivial,
so the output is

    proj = (((vision @ Wv.T + bv) @ Wiv.T + biv) @ Wo.T + bo) @ Wout.T + bout
    out  = LayerNorm(audio + proj[:, None, :]) * gamma + beta

Sharding: pure data parallel over batch (B=32 -> 4 rows per core, 8 cores).
All weights replicated; host pre-transposes them (W.T is what the PE wants
as the stationary operand) and slices the used third of in_proj.

Per-core device program:
  prologue: tiny PE matmul chain -> proj [4, 768] -> PE broadcast (ones
            outer-product) -> projB [128, 4, 768] in SBUF.
  main loop (64 tiles of [128 rows, 768]):
     DMA in -> DVE tensor_tensor_reduce (add projB + row-sum)
            -> ACT Square(x - mean) with accum_out (row sum of squares)
            -> ACT Sqrt / DVE reciprocal (rstd)
            -> DVE tensor_scalar (x - mean) * rstd  [-> * gamma + beta]
            -> DMA out.
The loop is DMA-bound (~48 MiB of HBM traffic per core); DVE/ACT are sized
to stay under the DMA roofline.
"""

import numpy as np

import concourse.bacc as bacc
import concourse.bass as bass
import concourse.mybir as mybir
import concourse.tile as tile
from concourse.bass_utils import run_bass_kernel_spmd

# Problem dims (hardcoded from the spec).
B, S, A, V, H = 32, 2048, 768, 512, 256
N_CORES = 8
BS = B // N_CORES          # 4 batch rows per core
P = 128                    # SBUF partitions
ROWS = BS * S              # 8192 rows per core
NT = ROWS // P             # 64 main-loop tiles
TPB = S // P               # 16 tiles per batch row
KV = V // P                # 4 k-tiles over the vision dim
KH = H // P                # 2 k-tiles over the hidden dim
MA = A // P                # 6 m-tiles over the audio dim
HALF = 384                 # matmul moving-free <= 512, so split A into 2
LN_EPS = 1e-5
F32 = mybir.dt.float32

_AF = mybir.ActivationFunctionType
_OP = mybir.AluOpType


def _build(apply_affine: bool) -> bass.Bass:
    nc = bacc.Bacc("TRN2", target_bir_lowering=False, debug=False, num_devices=N_CORES)

    audio = nc.dram_tensor("audio", [ROWS, A], F32, kind="ExternalInput").ap()
    visT = nc.dram_tensor("visT", [V, BS], F32, kind="ExternalInput").ap()
    wvT = nc.dram_tensor("wvT", [V, H], F32, kind="ExternalInput").ap()
    wivT = nc.dram_tensor("wivT", [H, H], F32, kind="ExternalInput").ap()
    woT = nc.dram_tensor("woT", [H, H], F32, kind="ExternalInput").ap()
    woutT = nc.dram_tensor("woutT", [H, A], F32, kind="ExternalInput").ap()
    bv = nc.dram_tensor("bv", [H], F32, kind="ExternalInput").ap()
    biv = nc.dram_tensor("biv", [H], F32, kind="ExternalInput").ap()
    bo = nc.dram_tensor("bo", [H], F32, kind="ExternalInput").ap()
    bout = nc.dram_tensor("bout", [A], F32, kind="ExternalInput").ap()
    if apply_affine:
        gamma = nc.dram_tensor("gamma", [A], F32, kind="ExternalInput").ap()
        beta = nc.dram_tensor("beta", [A], F32, kind="ExternalInput").ap()
    out = nc.dram_tensor("out", [ROWS, A], F32, kind="ExternalOutput").ap()

    def bcast_rows(ap_1d, parts):
        # DRAM [N] -> DMA-broadcast access pattern [parts, N] (partition stride 0)
        return bass.AP(tensor=ap_1d.tensor, offset=ap_1d.offset, ap=[[0, parts], *ap_1d.ap])

    with tile.TileContext(nc) as tc:
        with (
            tc.tile_pool(name="consts", bufs=1) as consts,
            tc.tile_pool(name="psum", bufs=2, space="PSUM") as psum,
            tc.tile_pool(name="xp", bufs=8) as xp,
            tc.tile_pool(name="sqp", bufs=4) as sqp,
            tc.tile_pool(name="stp", bufs=8) as stp,
        ):
            # ---- constants / weights ----
            ones = consts.tile([1, P], F32)
            nc.vector.memset(ones, 1.0)
            eps_sb = consts.tile([P, 1], F32)
            nc.vector.memset(eps_sb, LN_EPS)

            visT_sb = consts.tile([P, KV, BS], F32)
            nc.sync.dma_start(out=visT_sb, in_=visT.rearrange("(k p) b -> p k b", p=P))
            wvT_sb = consts.tile([P, KV, H], F32)
            nc.sync.dma_start(out=wvT_sb, in_=wvT.rearrange("(k p) h -> p k h", p=P))
            wivT_sb = consts.tile([P, KH, H], F32)
            nc.sync.dma_start(out=wivT_sb, in_=wivT.rearrange("(k p) h -> p k h", p=P))
            woT_sb = consts.tile([P, KH, H], F32)
            nc.sync.dma_start(out=woT_sb, in_=woT.rearrange("(k p) h -> p k h", p=P))
            woutT_sb = consts.tile([P, KH, A], F32)
            nc.sync.dma_start(out=woutT_sb, in_=woutT.rearrange("(k p) a -> p k a", p=P))

            bv_sb = consts.tile([P, KH], F32)
            nc.sync.dma_start(out=bv_sb, in_=bv.rearrange("(m p) -> p m", p=P))
            biv_sb = consts.tile([P, KH], F32)
            nc.sync.dma_start(out=biv_sb, in_=biv.rearrange("(m p) -> p m", p=P))
            bo_sb = consts.tile([P, KH], F32)
            nc.sync.dma_start(out=bo_sb, in_=bo.rearrange("(m p) -> p m", p=P))
            bout_sb = consts.tile([1, A], F32)
            nc.sync.dma_start(out=bout_sb, in_=bout.rearrange("(one a) -> one a", one=1))

            if apply_affine:
                gamma_sb = consts.tile([P, A], F32)
                nc.sync.dma_start(out=gamma_sb, in_=bcast_rows(gamma, P))
                beta_sb = consts.tile([P, A], F32)
                nc.sync.dma_start(out=beta_sb, in_=bcast_rows(beta, P))

            # ---- tiny projection chain, kept transposed: xT [P, k, BS] ----
            def chain_step(dst, wT_sb, n_k, bias_sb, xT):
                for mo in range(KH):
                    ps = psum.tile([P, BS], F32, tag="chain_ps")
                    for ki in range(n_k):
                        nc.tensor.matmul(
                            ps,
                            wT_sb[:, ki, mo * P : (mo + 1) * P],
                            xT[:, ki, :],
                            start=(ki == 0),
                            stop=(ki == n_k - 1),
                        )
                    nc.scalar.activation(
                        out=dst[:, mo, :], in_=ps, func=_AF.Identity,
                        bias=bias_sb[:, mo : mo + 1], scale=1.0,
                    )

            vT = consts.tile([P, KH, BS], F32)
            chain_step(vT, wvT_sb, KV, bv_sb, visT_sb)
            v2T = consts.tile([P, KH, BS], F32)
            chain_step(v2T, wivT_sb, KH, biv_sb, vT)
            attnT = consts.tile([P, KH, BS], F32)
            chain_step(attnT, woT_sb, KH, bo_sb, v2T)

            # proj rows, one per batch row, all at base partition 0:
            # proj[b] = attn[b] @ Wout.T + bout (bout folded in via a ones matmul)
            proj_rows = consts.tile([1, BS, A], F32)
            for b in range(BS):
                for h in range(A // HALF):
                    ps = psum.tile([1, HALF], F32, tag="proj_ps")
                    for ki in range(KH):
                        nc.tensor.matmul(
                            ps, attnT[:, ki, b : b + 1],
                            woutT_sb[:, ki, h * HALF : (h + 1) * HALF],
                            start=(ki == 0), stop=False,
                        )
                    nc.tensor.matmul(
                        ps, ones[:1, :1], bout_sb[:, h * HALF : (h + 1) * HALF],
                        start=False, stop=True,
                    )
                    nc.scalar.copy(out=proj_rows[:, b, h * HALF : (h + 1) * HALF], in_=ps)

            # broadcast each batch row across all 128 partitions via ones outer product
            projB = consts.tile([P, BS, A], F32)
            for b in range(BS):
                for h in range(A // HALF):
                    ps = psum.tile([P, HALF], F32, tag="bcast_ps")
                    nc.tensor.matmul(
                        ps, ones[:1, :P], proj_rows[:1, b, h * HALF : (h + 1) * HALF],
                        start=True, stop=True,
                    )
                    dst = projB[:, b, h * HALF : (h + 1) * HALF]
                    if (b + h) % 2 == 0:
                        nc.scalar.copy(out=dst, in_=ps)
                    else:
                        nc.vector.tensor_copy(out=dst, in_=ps)

            # ---- main loop: residual add + LayerNorm over 64 row-tiles ----
            for t in range(NT):
                b = t // TPB
                x = xp.tile([P, A], F32, tag="x")
                nc.sync.dma_start(out=x, in_=audio[t * P : (t + 1) * P, :])

                nc.vector.tensor_add(out=x, in0=x, in1=projB[:, b, :])

                # row sum via ACT copy-with-accumulate (ttr is broken on HW)
                d1 = sqp.tile([P, A], F32, tag="d1")
                sumx = stp.tile([P, 1], F32, tag="sumx")
                nc.scalar.activation(
                    out=d1, in_=x, func=_AF.Copy, bias=0.0, scale=1.0, accum_out=sumx,
                )
                negmean = stp.tile([P, 1], F32, tag="negmean")
                nc.vector.tensor_scalar_mul(out=negmean, in0=sumx, scalar1=-1.0 / A)

                sq = sqp.tile([P, A], F32, tag="sq")
                ssq = stp.tile([P, 1], F32, tag="ssq")
                nc.scalar.activation(
                    out=sq, in_=x, func=_AF.Square, bias=negmean, scale=1.0,
                    accum_out=ssq,
                )
                rstd = stp.tile([P, 1], F32, tag="rstd")
                nc.scalar.activation(
                    out=rstd, in_=ssq, func=_AF.Sqrt, bias=eps_sb, scale=1.0 / A,
                )
                nc.vector.reciprocal(out=rstd, in_=rstd)

                nc.vector.tensor_scalar(
                    out=x, in0=x, scalar1=negmean, scalar2=rstd,
                    op0=_OP.add, op1=_OP.mult,
                )
                if apply_affine:
                    nc.vector.tensor_mul(out=x, in0=x, in1=gamma_sb)
                    nc.vector.tensor_add(out=x, in0=x, in1=beta_sb)

                nc.sync.dma_start(out=out[t * P : (t + 1) * P, :], in_=x)

    nc.compile()
    return nc


_nc_cache: dict = {}


def _get_nc(apply_affine: bool) -> bass.Bass:
    if apply_affine not in _nc_cache:
        _nc_cache[apply_affine] = _build(apply_affine)
    return _nc_cache[apply_affine]


def make_in_maps(inputs: dict) -> tuple[list, bool]:
    """Host-side prep: slice batch per core, pre-transpose the tiny weights."""
    f = lambda k: np.ascontiguousarray(np.asarray(inputs[k]), dtype=np.float32)
    audio = f("audio_features")
    vision = f("vision_features")
    wvT = np.ascontiguousarray(f("Wv").T)
    wivT = np.ascontiguousarray(f("in_proj_w")[2 * H :].T)
    woT = np.ascontiguousarray(f("Wo_mha").T)
    woutT = np.ascontiguousarray(f("Wout").T)
    bv = f("bv")
    biv = np.ascontiguousarray(f("in_proj_b")[2 * H :])
    bo = f("bo_mha")
    bout = f("bout")
    gamma = f("gamma")
    beta = f("beta")
    apply_affine = not (np.all(gamma == 1.0) and np.all(beta == 0.0))

    in_maps = []
    for c in range(N_CORES):
        sl = slice(c * BS, (c + 1) * BS)
        m = {
            "audio": audio[sl].reshape(ROWS, A),
            "visT": np.ascontiguousarray(vision[sl].T),
            "wvT": wvT, "wivT": wivT, "woT": woT, "woutT": woutT,
            "bv": bv, "biv": biv, "bo": bo, "bout": bout,
        }
        if apply_affine:
            m["gamma"] = gamma
            m["beta"] = beta
        in_maps.append(m)
    return in_maps, apply_affine


def kernel(**inputs) -> np.ndarray:
    in_maps, apply_affine = make_in_maps(inputs)
    nc = _get_nc(apply_affine)
    res = run_bass_kernel_spmd(nc, in_maps, core_ids=list(range(N_CORES)))
    return np.concatenate(
        [r["out"].reshape(BS, S, A) for r in res.results], axis=0
    )


# revision 11
# speedup vs baseline: 42.3500x; 42.3500x over previous
"""Trainium2 Bass kernel for nn_CrossAttention_68350109549162.

Math (see reference): the single K/V token makes attention softmax trivial,
so the output is

    proj = (((vision @ Wv.T + bv) @ Wiv.T + biv) @ Wo.T + bo) @ Wout.T + bout
    out  = LayerNorm(audio + proj[:, None, :]) * gamma + beta

Sharding: pure data parallel over batch (B=32 -> 4 rows per core, 8 cores).
All weights replicated; host pre-transposes them (W.T is what the PE wants
as the stationary operand) and slices the used third of in_proj.

Per-core device program:
  prologue: tiny PE matmul chain -> proj [4, 768] -> PE broadcast (ones
            outer-product) -> projB [128, 4, 768] in SBUF.
  main loop (64 tiles of [128 rows, 768]):
     DMA in -> DVE tensor_tensor_reduce (add projB + row-sum)
            -> ACT Square(x - mean) with accum_out (row sum of squares)
            -> ACT Sqrt / DVE reciprocal (rstd)
            -> DVE tensor_scalar (x - mean) * rstd  [-> * gamma + beta]
            -> DMA out.
The loop is DMA-bound (~48 MiB of HBM traffic per core); DVE/ACT are sized
to stay under the DMA roofline.
"""

import numpy as np

import concourse.bacc as bacc
import concourse.bass as bass
import concourse.mybir as mybir
import concourse.tile as tile
from concourse.bass_utils import run_bass_kernel_spmd

# Problem dims (hardcoded from the spec).
B, S, A, V, H = 32, 2048, 768, 512, 256
N_CORES = 8
BS = B // N_CORES          # 4 batch rows per core
P = 128                    # SBUF partitions
ROWS = BS * S              # 8192 rows per core
NT = ROWS // P             # 64 main-loop tiles
TPB = S // P               # 16 tiles per batch row
KV = V // P                # 4 k-tiles over the vision dim
KH = H // P                # 2 k-tiles over the hidden dim
MA = A // P                # 6 m-tiles over the audio dim
HALF = 384                 # matmul moving-free <= 512, so split A into 2
LN_EPS = 1e-5
F32 = mybir.dt.float32

_AF = mybir.ActivationFunctionType
_OP = mybir.AluOpType


def _build(apply_affine: bool, n_reps: int = 1) -> bass.Bass:
    # n_reps > 1 repeats the main loop (same inputs/outputs) — used only by
    # test.py to measure steady-state HW time as a slope, immune to the
    # ~75 ms axon dispatch overhead. The graded path always uses n_reps=1.
    nc = bacc.Bacc("TRN2", target_bir_lowering=False, debug=False, num_devices=N_CORES)

    audio = nc.dram_tensor("audio", [ROWS, A], F32, kind="ExternalInput").ap()
    visT = nc.dram_tensor("visT", [V, BS], F32, kind="ExternalInput").ap()
    wvT = nc.dram_tensor("wvT", [V, H], F32, kind="ExternalInput").ap()
    wivT = nc.dram_tensor("wivT", [H, H], F32, kind="ExternalInput").ap()
    woT = nc.dram_tensor("woT", [H, H], F32, kind="ExternalInput").ap()
    woutT = nc.dram_tensor("woutT", [H, A], F32, kind="ExternalInput").ap()
    bv = nc.dram_tensor("bv", [H], F32, kind="ExternalInput").ap()
    biv = nc.dram_tensor("biv", [H], F32, kind="ExternalInput").ap()
    bo = nc.dram_tensor("bo", [H], F32, kind="ExternalInput").ap()
    bout = nc.dram_tensor("bout", [A], F32, kind="ExternalInput").ap()
    if apply_affine:
        gamma = nc.dram_tensor("gamma", [A], F32, kind="ExternalInput").ap()
        beta = nc.dram_tensor("beta", [A], F32, kind="ExternalInput").ap()
    out = nc.dram_tensor("out", [ROWS, A], F32, kind="ExternalOutput").ap()

    def bcast_rows(ap_1d, parts):
        # DRAM [N] -> DMA-broadcast access pattern [parts, N] (partition stride 0)
        return bass.AP(tensor=ap_1d.tensor, offset=ap_1d.offset, ap=[[0, parts], *ap_1d.ap])

    with tile.TileContext(nc) as tc:
        with (
            tc.tile_pool(name="consts", bufs=1) as consts,
            tc.tile_pool(name="psum", bufs=2, space="PSUM") as psum,
            tc.tile_pool(name="xp", bufs=8) as xp,
            tc.tile_pool(name="sqp", bufs=4) as sqp,
            tc.tile_pool(name="stp", bufs=8) as stp,
        ):
            # ---- constants / weights ----
            ones = consts.tile([1, P], F32)
            nc.vector.memset(ones, 1.0)
            eps_sb = consts.tile([P, 1], F32)
            nc.vector.memset(eps_sb, LN_EPS)

            visT_sb = consts.tile([P, KV, BS], F32)
            nc.sync.dma_start(out=visT_sb, in_=visT.rearrange("(k p) b -> p k b", p=P))
            wvT_sb = consts.tile([P, KV, H], F32)
            nc.sync.dma_start(out=wvT_sb, in_=wvT.rearrange("(k p) h -> p k h", p=P))
            wivT_sb = consts.tile([P, KH, H], F32)
            nc.sync.dma_start(out=wivT_sb, in_=wivT.rearrange("(k p) h -> p k h", p=P))
            woT_sb = consts.tile([P, KH, H], F32)
            nc.sync.dma_start(out=woT_sb, in_=woT.rearrange("(k p) h -> p k h", p=P))
            woutT_sb = consts.tile([P, KH, A], F32)
            nc.sync.dma_start(out=woutT_sb, in_=woutT.rearrange("(k p) a -> p k a", p=P))

            bv_sb = consts.tile([P, KH], F32)
            nc.sync.dma_start(out=bv_sb, in_=bv.rearrange("(m p) -> p m", p=P))
            biv_sb = consts.tile([P, KH], F32)
            nc.sync.dma_start(out=biv_sb, in_=biv.rearrange("(m p) -> p m", p=P))
            bo_sb = consts.tile([P, KH], F32)
            nc.sync.dma_start(out=bo_sb, in_=bo.rearrange("(m p) -> p m", p=P))
            bout_sb = consts.tile([1, A], F32)
            nc.sync.dma_start(out=bout_sb, in_=bout.rearrange("(one a) -> one a", one=1))

            if apply_affine:
                gamma_sb = consts.tile([P, A], F32)
                nc.sync.dma_start(out=gamma_sb, in_=bcast_rows(gamma, P))
                beta_sb = consts.tile([P, A], F32)
                nc.sync.dma_start(out=beta_sb, in_=bcast_rows(beta, P))

            # ---- tiny projection chain, kept transposed: xT [P, k, BS] ----
            def chain_step(dst, wT_sb, n_k, bias_sb, xT):
                for mo in range(KH):
                    ps = psum.tile([P, BS], F32, tag="chain_ps")
                    for ki in range(n_k):
                        nc.tensor.matmul(
                            ps,
                            wT_sb[:, ki, mo * P : (mo + 1) * P],
                            xT[:, ki, :],
                            start=(ki == 0),
                            stop=(ki == n_k - 1),
                        )
                    nc.scalar.activation(
                        out=dst[:, mo, :], in_=ps, func=_AF.Identity,
                        bias=bias_sb[:, mo : mo + 1], scale=1.0,
                    )

            vT = consts.tile([P, KH, BS], F32)
            chain_step(vT, wvT_sb, KV, bv_sb, visT_sb)
            v2T = consts.tile([P, KH, BS], F32)
            chain_step(v2T, wivT_sb, KH, biv_sb, vT)
            attnT = consts.tile([P, KH, BS], F32)
            chain_step(attnT, woT_sb, KH, bo_sb, v2T)

            # proj rows, one per batch row, all at base partition 0:
            # proj[b] = attn[b] @ Wout.T + bout (bout folded in via a ones matmul)
            proj_rows = consts.tile([1, BS, A], F32)
            for b in range(BS):
                for h in range(A // HALF):
                    ps = psum.tile([1, HALF], F32, tag="proj_ps")
                    for ki in range(KH):
                        nc.tensor.matmul(
                            ps, attnT[:, ki, b : b + 1],
                            woutT_sb[:, ki, h * HALF : (h + 1) * HALF],
                            start=(ki == 0), stop=False,
                        )
                    nc.tensor.matmul(
                        ps, ones[:1, :1], bout_sb[:, h * HALF : (h + 1) * HALF],
                        start=False, stop=True,
                    )
                    nc.scalar.copy(out=proj_rows[:, b, h * HALF : (h + 1) * HALF], in_=ps)

            # broadcast each batch row across all 128 partitions via ones outer product
            projB = consts.tile([P, BS, A], F32)
            for b in range(BS):
                for h in range(A // HALF):
                    ps = psum.tile([P, HALF], F32, tag="bcast_ps")
                    nc.tensor.matmul(
                        ps, ones[:1, :P], proj_rows[:1, b, h * HALF : (h + 1) * HALF],
                        start=True, stop=True,
                    )
                    dst = projB[:, b, h * HALF : (h + 1) * HALF]
                    if (b + h) % 2 == 0:
                        nc.scalar.copy(out=dst, in_=ps)
                    else:
                        nc.vector.tensor_copy(out=dst, in_=ps)

            # ---- main loop: residual add + LayerNorm over 64 row-tiles ----
            import contextlib

            rep_ctx = (
                tc.For_i(
                    0, n_reps, 1,
                    hint_engines=(
                        mybir.EngineType.DVE,
                        mybir.EngineType.Activation,
                        mybir.EngineType.SP,
                    ),
                )
                if n_reps > 1
                else contextlib.nullcontext()
            )
            with rep_ctx:
              for t in range(NT):
                b = t // TPB
                x = xp.tile([P, A], F32, tag="x")
                nc.sync.dma_start(out=x, in_=audio[t * P : (t + 1) * P, :])

                nc.vector.tensor_add(out=x, in0=x, in1=projB[:, b, :])

                # row sum via ACT copy-with-accumulate (ttr is broken on HW)
                d1 = sqp.tile([P, A], F32, tag="d1")
                sumx = stp.tile([P, 1], F32, tag="sumx")
                nc.scalar.activation(
                    out=d1, in_=x, func=_AF.Copy, bias=0.0, scale=1.0, accum_out=sumx,
                )
                negmean = stp.tile([P, 1], F32, tag="negmean")
                nc.vector.tensor_scalar_mul(out=negmean, in0=sumx, scalar1=-1.0 / A)

                sq = sqp.tile([P, A], F32, tag="sq")
                ssq = stp.tile([P, 1], F32, tag="ssq")
                nc.scalar.activation(
                    out=sq, in_=x, func=_AF.Square, bias=negmean, scale=1.0,
                    accum_out=ssq,
                )
                rstd = stp.tile([P, 1], F32, tag="rstd")
                nc.scalar.activation(
                    out=rstd, in_=ssq, func=_AF.Sqrt, bias=eps_sb, scale=1.0 / A,
                )
                nc.vector.reciprocal(out=rstd, in_=rstd)

                nc.vector.tensor_scalar(
                    out=x, in0=x, scalar1=negmean, scalar2=rstd,
                    op0=_OP.add, op1=_OP.mult,
                )
                if apply_affine:
                    nc.vector.tensor_mul(out=x, in0=x, in1=gamma_sb)
                    nc.vector.tensor_add(out=x, in0=x, in1=beta_sb)

                nc.sync.dma_start(out=out[t * P : (t + 1) * P, :], in_=x)

    nc.compile()
    return nc


_nc_cache: dict = {}


def _get_nc(apply_affine: bool, n_reps: int = 1) -> bass.Bass:
    key = (apply_affine, n_reps)
    if key not in _nc_cache:
        _nc_cache[key] = _build(apply_affine, n_reps)
    return _nc_cache[key]


def make_in_maps(inputs: dict) -> tuple[list, bool]:
    """Host-side prep: slice batch per core, pre-transpose the tiny weights."""
    f = lambda k: np.ascontiguousarray(np.asarray(inputs[k]), dtype=np.float32)
    audio = f("audio_features")
    vision = f("vision_features")
    wvT = np.ascontiguousarray(f("Wv").T)
    wivT = np.ascontiguousarray(f("in_proj_w")[2 * H :].T)
    woT = np.ascontiguousarray(f("Wo_mha").T)
    woutT = np.ascontiguousarray(f("Wout").T)
    bv = f("bv")
    biv = np.ascontiguousarray(f("in_proj_b")[2 * H :])
    bo = f("bo_mha")
    bout = f("bout")
    gamma = f("gamma")
    beta = f("beta")
    apply_affine = not (np.all(gamma == 1.0) and np.all(beta == 0.0))

    in_maps = []
    for c in range(N_CORES):
        sl = slice(c * BS, (c + 1) * BS)
        m = {
            "audio": audio[sl].reshape(ROWS, A),
            "visT": np.ascontiguousarray(vision[sl].T),
            "wvT": wvT, "wivT": wivT, "woT": woT, "woutT": woutT,
            "bv": bv, "biv": biv, "bo": bo, "bout": bout,
        }
        if apply_affine:
            m["gamma"] = gamma
            m["beta"] = beta
        in_maps.append(m)
    return in_maps, apply_affine


def kernel(**inputs) -> np.ndarray:
    in_maps, apply_affine = make_in_maps(inputs)
    nc = _get_nc(apply_affine)
    res = run_bass_kernel_spmd(nc, in_maps, core_ids=list(range(N_CORES)))
    return np.concatenate(
        [r["out"].reshape(BS, S, A) for r in res.results], axis=0
    )
